# revision 104
# baseline (speedup 1.0000x reference)
"""Trainium2 Bass kernel for CrossMultiheadAttention.

B=4, T=S=1024, E=1024, H=16, D=64. 8 NeuronCores.

Sharding: core c handles (batch b=c//2, T-half th=c%2) -> 512 query rows.
Each core computes k/v projections for its whole batch (duplicated between
the 2 cores sharing a batch), all 16 heads of attention for its queries and
the full output projection for its rows. Output gather is a pure concat.

Single fused pipeline engineered to keep the PE array continuously busy
(TRN2 p-state: full 2.4 GHz clock only after ~3us of gapless execution):
the attention j-loop for head-pair hp is interleaved with the q/k/v
projections of hp+1 so the PE always has independent matmuls to run while
the DVE(bias-add, in-place in PSUM) -> ACT(exp) chain produces e-tiles.
attn_bias streams as bf16 with one batched DMA per (hp, head, j-half).
Per-head-pair softmax normalization stays on-chip: the two denominators
(65th v-column trick) are reciprocal'd on DVE in-lane, broadcast across
partitions with two K=1 matmuls, and multiplied into oT on GpSimd directly
from PSUM. The output projection runs as a short tail.
"""
import sys

sys.path.insert(0, "/opt/trn_rl_repo")

import numpy as np
import ml_dtypes

import concourse.bass as bass
import concourse.bacc as bacc
import concourse.tile as tile
from concourse import mybir
from concourse.bass_utils import run_bass_kernel_spmd


def _pbcast(ap, nparts):
    """View `ap` (a [1, N] row) replicated across nparts partitions via a
    0-stride partition dim — DMA-source only."""
    row = ap
    return bass.AP(tensor=row.tensor, offset=row.offset,
                   ap=[[0, nparts]] + [list(d) for d in row.ap[1:]])


F32 = mybir.dt.float32
BF16 = mybir.dt.bfloat16
Act = mybir.ActivationFunctionType
Alu = mybir.AluOpType
NPBF16 = ml_dtypes.bfloat16

B, T, S, E, H, D = 4, 1024, 1024, 1024, 16, 64
HP = H // 2          # head pairs
TS = T // 2          # per-core query rows (t-shard)
ET = E // 128        # 128-row tiles of the embed dim
SCALING = D ** -0.5
MASK_NEG = -10000.0

_CACHE = {}


def build_nc():
    nc = bacc.Bacc("TRN2", target_bir_lowering=False, debug=False, num_devices=8)

    qin_d = nc.dram_tensor("qin", [E, TS], BF16, kind="ExternalInput").ap()
    kin_d = nc.dram_tensor("kin", [E, S], BF16, kind="ExternalInput").ap()
    vin_d = nc.dram_tensor("vin", [E, S], BF16, kind="ExternalInput").ap()
    # expb = exp(attn_bias + key_padding_mask*-inf), transposed [H, S, TS]
    bias_d = nc.dram_tensor("expb", [H, S, TS], BF16, kind="ExternalInput").ap()
    wq_d = nc.dram_tensor("wqt", [E, E], BF16, kind="ExternalInput").ap()
    wk_d = nc.dram_tensor("wkt", [E, E], BF16, kind="ExternalInput").ap()
    # wv augmented: per head 64 wv columns + 1 zero column (ones slot)
    wv_d = nc.dram_tensor("wvta", [E, H * 65], BF16, kind="ExternalInput").ap()
    wo_d = nc.dram_tensor("wot", [E, E], BF16, kind="ExternalInput").ap()
    bq_d = nc.dram_tensor("bqs", [128, 8], F32, kind="ExternalInput").ap()
    bk_d = nc.dram_tensor("bks", [128, 8], F32, kind="ExternalInput").ap()
    # per head: 64 bv values + literal 1.0 (fills the ones column)
    bv_d = nc.dram_tensor("bvr65", [1, H * 65], BF16, kind="ExternalInput").ap()
    bo_d = nc.dram_tensor("borb", [1, E], BF16, kind="ExternalInput").ap()
    out_d = nc.dram_tensor("out", [TS, E], F32, kind="ExternalOutput").ap()

    with tile.TileContext(nc) as tc:
        with tc.tile_pool(name="consts", bufs=1) as consts, \
             tc.tile_pool(name="wpool", bufs=1) as wpool, \
             tc.tile_pool(name="in2k", bufs=1) as in2k, \
             tc.tile_pool(name="qpool", bufs=1) as qpool, \
             tc.tile_pool(name="persist", bufs=1) as persist, \
             tc.tile_pool(name="bpool", bufs=12) as bpool, \
             tc.tile_pool(name="estream", bufs=10) as estream, \
             tc.tile_pool(name="eib", bufs=6) as eib, \
             tc.tile_pool(name="osbp", bufs=2) as osbp, \
             tc.tile_pool(name="pscore", bufs=2, space="PSUM") as pscore, \
             tc.tile_pool(name="ppot", bufs=2, space="PSUM") as ppot, \
             tc.tile_pool(name="pproj", bufs=2, space="PSUM") as pproj:

            # ---- constants ----
            bq_sb = consts.tile([128, 8], F32, tag="bq")
            bk_sb = consts.tile([128, 8], F32, tag="bk")
            bvrow = consts.tile([1, H * 65], BF16, tag="bvrow")
            borow = consts.tile([1, E], BF16, tag="borow")  # bo row (bf16)
            ones1 = consts.tile([1, 128], BF16, tag="ones1")
            selAB = consts.tile([1, 256], BF16, tag="selAB")
            selA = selAB[:, 0:128]
            selB = selAB[:, 128:256]
            nc.scalar.dma_start(out=bq_sb, in_=bq_d)
            nc.scalar.dma_start(out=bk_sb, in_=bk_d)
            nc.scalar.dma_start(out=bvrow, in_=bv_d)
            nc.scalar.dma_start(out=borow, in_=bo_d)
            nc.vector.memset(ones1, 1.0)
            nc.vector.memset(selA, 0.0)
            nc.vector.memset(selA[:, 0:64], 1.0)
            nc.vector.memset(selB, 0.0)
            nc.vector.memset(selB[:, 64:128], 1.0)
            # warm the Exp activation table during the DMA-bound prologue
            # (otherwise the first exp pays a ~1.3us table load mid-pipeline)
            warm = consts.tile([1, 8], F32, tag="warm")
            nc.scalar.activation(warm, ones1[:, 0:8], Act.Exp)

            # ---- persistent SBUF tensors ----
            qT = [persist.tile([128, TS], BF16, tag=f"qt{hp}", name=f"qt{hp}")
                  for hp in range(HP)]
            kT = [[persist.tile([128, 512], BF16, tag=f"kt{hp}_{sh}",
                                name=f"kt{hp}_{sh}")
                   for sh in range(2)] for hp in range(HP)]
            # v65[j]: per s-tile j, per head h a 65-wide block: cols 0..63 =
            # d, col 64 = ones (gives the softmax denominator for free as
            # row 64 of the o-matmul PSUM accumulation).
            v65 = [persist.tile([128, H, 65], BF16, tag=f"v65_{j}",
                                name=f"v65_{j}")
                   for j in range(8)]
            otn2 = [persist.tile([128, TS], BF16, tag=f"otn2_{et}",
                                 name=f"otn2_{et}")
                    for et in range(ET)]
            # denominator rows, ACT-copied (with partition shift) to lane 0
            den_f32 = [[persist.tile([1, TS], F32, tag=f"df{hh}_{i}",
                                     name=f"df{hh}_{i}")
                        for hh in range(2)] for i in range(2)]
            rcp_f32 = [[persist.tile([1, TS], F32, tag=f"rf{hh}_{i}",
                                     name=f"rf{hh}_{i}")
                        for hh in range(2)] for i in range(2)]
            rcp_sb = [[persist.tile([1, TS], BF16, tag=f"rcp{hh}_{i}",
                                    name=f"rcp{hh}_{i}")
                       for hh in range(2)] for i in range(2)]
            bc_sb = [persist.tile([128, TS], BF16, tag=f"bc_{i}",
                                  name=f"bc_{i}")
                     for i in range(2)]

            # ---- weights + inputs: ONE consolidated DMA per tensor (the
            # per-DMA DGE config cost would otherwise serialize the
            # prologue). [E, X] dram -> [128, ET*X] sbuf, et-tiles along
            # the free dim.
            def big_load(dram, pool, tag, cols, nsplit=4, eng=None, head=0):
                # split into nsplit DMAs so transfers parallelize across
                # DMA queues; issue from the given sequencer (spreading the
                # per-DMA DGE config cost across idle engines). `head`
                # pulls that many leading columns of every et-tile first so
                # the first consumer unblocks sooner.
                t_ = pool.tile([128, ET * cols], BF16, tag=tag, name=tag)
                src = dram.rearrange("(et p) x -> p et x", p=128)
                dst = t_.rearrange("p (et x) -> p et x", et=ET)
                e_ = eng or nc.sync
                if head:
                    e_.dma_start(out=dst[:, :, 0:head], in_=src[:, :, 0:head])

                    def rest(e2=None):
                        (e2 or e_).dma_start(out=dst[:, :, head:cols],
                                             in_=src[:, :, head:cols])
                    return t_, rest
                step = ET // nsplit
                for i in range(nsplit):
                    e_.dma_start(
                        out=dst[:, i * step:(i + 1) * step, :],
                        in_=src[:, i * step:(i + 1) * step, :])
                return t_

            # critical-path chunks first (k/q/v for hp0, s-half 0); the
            # rest streams during hp0's j-loop
            # longest-pole transfers first on each sequencer: kin-head and
            # qin gate k(0,0) / q(0)
            kin_big, kin_rest = big_load(kin_d, in2k, "kin", S, head=512)
            wk_big, wk_rest = big_load(wk_d, wpool, "wk", E, head=256)
            qin_big = big_load(qin_d, qpool, "qin", TS, nsplit=2,
                               eng=nc.scalar)
            wq_big, wq_rest = big_load(wq_d, wpool, "wq", E, eng=nc.scalar,
                                       head=256)
            wv_big, wv_rest = big_load(wv_d, wpool, "wv", H * 65,
                                       eng=nc.gpsimd, head=260)
            vin_big, vin_rest = big_load(vin_d, in2k, "vin", S,
                                         eng=nc.gpsimd, head=512)
            wk_sb = [wk_big[:, et * E:(et + 1) * E] for et in range(ET)]
            wq_sb = [wq_big[:, et * E:(et + 1) * E] for et in range(ET)]
            wv_sb = [wv_big[:, et * H * 65:(et + 1) * H * 65]
                     for et in range(ET)]
            key_sb = [kin_big[:, et * S:(et + 1) * S] for et in range(ET)]
            val_sb = [vin_big[:, et * S:(et + 1) * S] for et in range(ET)]
            qin_sb = [qin_big[:, et * TS:(et + 1) * TS] for et in range(ET)]
            # wo is loaded late (hp6) into the kin buffer — see wo_load()
            wo_sb = []

            def wo_load():
                w = big_load(wo_d, in2k, "kin", E)
                wo_sb.extend(w[:, et * E:(et + 1) * E] for et in range(ET))

            # ---- bias streaming: per (hp, hh, j-quarter) one DMA of 2
            # j-tiles; dest [128, 2*512] bf16, src 3D AP over biasT
            bias_tiles = {}

            def bias_dma(hp, hh, jq, eng=None):
                h = 2 * hp + hh
                bt = bpool.tile([128, 2 * TS], BF16, tag="bias", name="bias")
                src = bias_d[h, jq * 256:(jq + 1) * 256, :].rearrange(
                    "(j p) t -> p j t", p=128)
                (eng or nc.sync).dma_start(
                    out=bt.rearrange("p (j t) -> p j t", t=TS), in_=src)
                bias_tiles[(hp, hh, jq)] = bt

            def bias_slice(hp, hh, j):
                bt = bias_tiles[(hp, hh, j // 2)]
                return bt[:, (j % 2) * TS:(j % 2 + 1) * TS]

            # only the first two j-quarters of hp0's bias up front; the
            # rest goes into hp0's fill (frees early HBM bandwidth for the
            # projection inputs)
            for jq in range(2):
                for hh in range(2):
                    bias_dma(0, hh, jq)

            # ---- projection emitters (one PSUM-group unit each) ----
            def k_unit(hp, sh):
                ps = pproj.tile([128, 512], F32, tag="proj", name="psp")
                for et in range(ET):
                    nc.tensor.matmul(ps, wk_sb[et][:, hp * 128:(hp + 1) * 128],
                                     key_sb[et][:, sh * 512:(sh + 1) * 512],
                                     start=(et == 0), stop=(et == ET - 1))
                nc.vector.tensor_scalar(out=kT[hp][sh], in0=ps,
                                        scalar1=bk_sb[:, hp:hp + 1],
                                        scalar2=None, op0=Alu.add)

            def q_unit(hp):
                ps = pproj.tile([128, 512], F32, tag="proj", name="psp")
                for et in range(ET):
                    nc.tensor.matmul(ps, wq_sb[et][:, hp * 128:(hp + 1) * 128],
                                     qin_sb[et], start=(et == 0),
                                     stop=(et == ET - 1))
                nc.scalar.activation(qT[hp], ps, Act.Identity,
                                     bias=bq_sb[:, hp:hp + 1])

            def v_unit(grp, st):
                # heads 4*grp .. 4*grp+3, s-block st -> v65[st][:, 4 heads].
                # wv is host-augmented with zero columns in the per-head
                # ones slots; the K=1 bv row writes bv plus literal 1.0
                # there, so the PSUM holds finished v65 (contiguous copy).
                ps = pproj.tile([128, 512], F32, tag="proj", name="psp")
                c0 = grp * 260
                for et in range(ET):
                    nc.tensor.matmul(ps[:, 0:260],
                                     val_sb[et][:, st * 128:(st + 1) * 128],
                                     wv_sb[et][:, c0:c0 + 260],
                                     start=(et == 0), stop=False)
                nc.tensor.matmul(ps[:, 0:260], ones1, bvrow[:, c0:c0 + 260],
                                 start=False, stop=True)
                h0 = 4 * grp
                nc.scalar.copy(
                    out=v65[st][:, h0:h0 + 4, :],
                    in_=ps[:, 0:260].rearrange("p (h d) -> p h d", d=65))

            # ---- prologue: hp0 k/q projections; v units and the
            # remaining loads go into hp0's fill so the attention j-loop
            # starts as soon as kT/qT/bias land. kin_rest is issued here —
            # the SP sequencer runs ahead of the PE — so k(0,1) can fill
            # the early DMA-wait hole.
            k_unit(0, 0)
            q_unit(0)
            kin_rest()
            k_unit(0, 1)
            v_unit(0, 0)
            v_unit(0, 1)
            v_unit(0, 2)
            v_unit(0, 3)

            # ---- out-projection group helpers (hp7 early-start + tail)
            og_ps = {}

            def og_mm(g, ps, ets):
                tt, ih = g // 2, g % 2
                for et in ets:
                    nc.tensor.matmul(
                        ps, otn2[et][:, tt * 128:(tt + 1) * 128],
                        wo_sb[et][:, ih * 512:(ih + 1) * 512],
                        start=(et == 0), stop=False)

            def og_start(g):
                # early groups accumulate during hp7 / the normalize-7
                # chain; pool choice avoids circular waits: g0 pproj,
                # g1/g2 pscore, g3 ppot (pot1 slot freed by normalize(6))
                if g == 0:
                    ps = pproj.tile([128, 512], F32, tag="proj", name="psp")
                elif g == 3:
                    ps = ppot.tile([128, 512], F32, tag="pot1", name="pot1")
                else:
                    ps = pscore.tile([128, 512], F32, tag="sc", name="sc")
                og_ps[g] = ps
                og_mm(g, ps, range(6) if g < 2 else range(7))

            # ---- fused attention + interleaved projections ----
            ebuf = {}

            def score_pair(hp, j):
                sh, sl = j // 4, j % 4
                pss = [pscore.tile([128, 512], F32, tag="sc", name="sc")
                       for _ in range(2)]
                for hh in range(2):
                    nc.tensor.matmul(
                        pss[hh],
                        kT[hp][sh][hh * 64:(hh + 1) * 64,
                                   sl * 128:(sl + 1) * 128],
                        qT[hp][hh * 64:(hh + 1) * 64, :],
                        start=True, stop=True,
                        tile_position=(hh * 64, 0))
                return pss

            nmul = [0]

            def bias_exp(hp, j, pss):
                # e = exp(scores) * exp(bias+mask): the exp(bias) factor is
                # host-precomputed, so the elementwise combine is an
                # SBUF-only bf16 multiply that DVE and GpSimd can share.
                for hh in range(2):
                    e0 = eib.tile([128, TS], BF16, tag="ei", name="ei")
                    nc.scalar.activation(e0, pss[hh], Act.Exp)
                    e_ = estream.tile([128, TS], BF16, tag="e", name="e")
                    eng = nc.vector  # bisect: no gpsimd
                    nmul[0] += 1
                    eng.tensor_tensor(out=e_, in0=e0,
                                      in1=bias_slice(hp, hh, j),
                                      op=Alu.mult)
                    ebuf[(j, hh)] = e_

            def o_mm(hp, j, poT):
                for hh in range(2):
                    h = 2 * hp + hh
                    nc.tensor.matmul(poT[hh][0:65, :], v65[j][:, h, :],
                                     ebuf.pop((j, hh)),
                                     start=(j == 0), stop=(j == 7))

            def normalize_rcp(hp, poT):
                # denominators sit on row 64 of each poT (ones-col trick);
                # ACT copy shifts them to lane 0, then fast approx recip.
                i = hp % 2
                for hh in range(2):
                    nc.scalar.copy(out=den_f32[i][hh], in_=poT[hh][64:65, :])
                    nc.vector.reciprocal_approx_fast(out=rcp_f32[i][hh],
                                                     in_=den_f32[i][hh])
                    nc.vector.tensor_copy(out=rcp_sb[i][hh],
                                          in_=rcp_f32[i][hh])

            def normalize_mul(hp, poT, final=False):
                # two K=1 matmuls broadcast 1/den across partitions
                # 0-63 / 64-127, then DVE scales oT into bf16 otn2.
                # The final call draws its PSUM from ppot (pproj/pscore are
                # held by the early out-projection groups then).
                i = hp % 2
                if final:
                    bc = ppot.tile([128, 512], F32, tag="pot0", name="pot0")
                else:
                    bc = pproj.tile([128, 512], F32, tag="proj", name="psp")
                nc.tensor.matmul(bc, selA, rcp_sb[i][0], start=True,
                                 stop=False)
                nc.tensor.matmul(bc, selB, rcp_sb[i][1], start=False,
                                 stop=True)
                nc.scalar.copy(out=bc_sb[i], in_=bc)
                nc.vector.tensor_tensor(out=otn2[hp][0:64, :],
                                        in0=poT[0][0:64, :],
                                        in1=bc_sb[i][0:64, :],
                                        op=Alu.mult)
                nc.vector.tensor_tensor(out=otn2[hp][64:128, :],
                                        in0=poT[1][0:64, :],
                                        in1=bc_sb[i][64:128, :],
                                        op=Alu.mult)

            prev = None  # (hp, poT) awaiting normalize
            for hp in range(HP):
                # fill work: projections + bias prefetch for hp+1, emitted
                # between j-steps so the PE always has independent matmuls
                fill = []
                if hp == 0:
                    for jq in range(2, 4):
                        for hh in range(2):
                            fill.append(
                                lambda hh=hh, jq=jq: bias_dma(0, hh, jq))

                    fill.append(lambda: vin_rest())
                    fill.append(lambda: wk_rest())
                    fill.append(lambda: wq_rest(nc.scalar))
                    fill.append(lambda: wv_rest())
                    for st in range(4, 8):
                        fill.append(lambda st=st: v_unit(0, st))
                if hp + 1 < HP:
                    fill.append(lambda n=hp + 1: k_unit(n, 0))
                    fill.append(lambda n=hp + 1: k_unit(n, 1))
                    fill.append(lambda n=hp + 1: q_unit(n))
                    if hp < 6:
                        g = 1 + hp // 2
                        # grp3's last two s-blocks shift to hp6 (its fill
                        # is otherwise light); they complete well before
                        # hp6's own o_mm consumes those heads
                        if hp == 5:
                            sts = range(4, 6)
                        else:
                            sts = range(0, 4) if hp % 2 == 0 else range(4, 8)
                        for st in sts:
                            fill.append(lambda g=g, st=st: v_unit(g, st))
                    for jq in range(4):
                        for hh in range(2):
                            fill.append(
                                lambda n=hp + 1, hh=hh, jq=jq: bias_dma(n, hh, jq))
                if hp == 6:
                    fill.insert(0, lambda: v_unit(3, 7))
                    fill.insert(0, lambda: v_unit(3, 6))
                    fill.append(wo_load)
                if hp == 7:
                    fill.append(lambda: og_start(0))
                    fill.append(lambda: og_start(1))


                poT = [ppot.tile([128, 512], F32, tag=f"pot{k}", name=f"pot{k}")
                       for k in range(2)]
                nf = len(fill)
                fi = 0
                for j in range(8):
                    pss = score_pair(hp, j)
                    bias_exp(hp, j, pss)
                    if prev is not None:
                        if j == 2:
                            normalize_rcp(*prev)
                        elif j == 5:
                            normalize_mul(*prev)
                    if j >= 2:
                        o_mm(hp, j - 2, poT)
                    # spread fill units across the 8 j-steps
                    want = (j + 1) * nf // 8
                    while fi < want:
                        fill[fi]()
                        fi += 1
                o_mm(hp, 6, poT)
                o_mm(hp, 7, poT)
                prev = (hp, poT)
            normalize_rcp(*prev)
            # keep the PE streaming through the reciprocal chain: groups
            # 2/3 accumulate et 0..6 (otn2[6] settled since hp7 j5)
            og_start(2)
            og_start(3)
            normalize_mul(*prev, final=True)

            # ---- output projection tail: groups 0-1 were started during
            # hp7 (et 0..6); finish them and run groups 2-7 over 4 PSUM
            # banks (pproj + the now-idle pscore pool). bo folds in as a
            # K=1 ones-row, ACT evacuates, DMA out.
            for g in range(8):
                tt, ih = g // 2, g % 2
                if g < 2:
                    ps = og_ps.pop(g)
                    og_mm(g, ps, range(6, ET))
                elif g < 4:
                    ps = og_ps.pop(g)
                    og_mm(g, ps, range(7, ET))
                else:
                    if g % 2 == 0:
                        ps = pproj.tile([128, 512], F32, tag="proj",
                                        name="psp")
                    else:
                        ps = pscore.tile([128, 512], F32, tag="sc",
                                         name="sc")
                    og_mm(g, ps, range(ET))
                nc.tensor.matmul(ps, ones1,
                                 borow[:, ih * 512:(ih + 1) * 512],
                                 start=False, stop=True)
                o = osbp.tile([128, 512], F32, tag="osb", name="osb")
                if g % 2 == 0:
                    nc.scalar.copy(out=o, in_=ps)
                else:
                    nc.vector.tensor_copy(out=o, in_=ps)
                nc.sync.dma_start(
                    out=out_d[tt * 128:(tt + 1) * 128,
                              ih * 512:(ih + 1) * 512],
                    in_=o)

    nc.compile()
    return nc


def _prepare_in_maps(query, key, value, key_padding_mask, attn_bias,
                     wq, bq, wk, bk, wv, bv, wo, bo):
    wqt = (np.ascontiguousarray(wq.T) * SCALING).astype(NPBF16)
    wkt = np.ascontiguousarray(wk.T).astype(NPBF16)
    wot = np.ascontiguousarray(wo.T).astype(NPBF16)
    # wv augmented with a zero column per head (the v65 ones slot)
    wvta = np.zeros((E, H * 65), dtype=NPBF16)
    wvta_v = wvta.reshape(E, H, 65)
    wvta_v[:, :, 0:64] = wv.T.reshape(E, H, 64)
    bvr65 = np.ones((1, H * 65), dtype=NPBF16)
    bvr65.reshape(H, 65)[:, 0:64] = np.asarray(bv, np.float32).reshape(H, 64)
    bqs = np.ascontiguousarray((bq * SCALING).reshape(8, 128).T)
    bks = np.ascontiguousarray(bk.astype(np.float32).reshape(8, 128).T)
    borb = np.ascontiguousarray(bo)[None, :].astype(NPBF16)

    kin_b = [np.ascontiguousarray(key[b_].T).astype(NPBF16) for b_ in range(B)]
    vin_b = [np.ascontiguousarray(value[b_].T).astype(NPBF16) for b_ in range(B)]
    maskadd = [
        np.where(key_padding_mask[b_], MASK_NEG, 0.0).astype(np.float32)
        for b_ in range(B)
    ]

    in_maps = []
    for c in range(8):
        b_, th = c // 2, c % 2
        qin = np.ascontiguousarray(
            query[b_, th * TS:(th + 1) * TS, :].T).astype(NPBF16)
        biasT = np.ascontiguousarray(
            attn_bias[b_ * H:(b_ + 1) * H, th * TS:(th + 1) * TS, :]
            .transpose(0, 2, 1))
        expb = np.exp(biasT + maskadd[b_][None, :, None]).astype(NPBF16)
        in_maps.append({
            "qin": qin, "kin": kin_b[b_], "vin": vin_b[b_],
            "expb": expb,
            "wqt": wqt, "wkt": wkt, "wvta": wvta, "wot": wot,
            "bqs": bqs, "bks": bks, "bvr65": bvr65, "borb": borb,
        })
    return in_maps


def kernel(query, key, value, key_padding_mask, attn_bias,
           wq, bq, wk, bk, wv, bv, wo, bo, _run_kwargs=None):
    query = np.asarray(query, dtype=np.float32)
    key = np.asarray(key, dtype=np.float32)
    value = np.asarray(value, dtype=np.float32)
    key_padding_mask = np.asarray(key_padding_mask)
    attn_bias = np.asarray(attn_bias, dtype=np.float32)
    wq, bq = np.asarray(wq, np.float32), np.asarray(bq, np.float32)
    wk, bk = np.asarray(wk, np.float32), np.asarray(bk, np.float32)
    wv, bv = np.asarray(wv, np.float32), np.asarray(bv, np.float32)
    wo, bo = np.asarray(wo, np.float32), np.asarray(bo, np.float32)

    if "nc" not in _CACHE:
        _CACHE["nc"] = build_nc()
    nc = _CACHE["nc"]

    in_maps = _prepare_in_maps(query, key, value, key_padding_mask, attn_bias,
                               wq, bq, wk, bk, wv, bv, wo, bo)
    res = run_bass_kernel_spmd(nc, in_maps, core_ids=list(range(8)),
                               **(_run_kwargs or {}))
    _CACHE["last_results"] = res

    out = np.empty((B, T, E), dtype=np.float32)
    for c in range(8):
        b_, th = c // 2, c % 2
        out[b_, th * TS:(th + 1) * TS, :] = res.results[c]["out"]
    return out


# revision 107
# speedup vs baseline: 1.0049x; 1.0049x over previous
"""Trainium2 Bass kernel for CrossMultiheadAttention.

B=4, T=S=1024, E=1024, H=16, D=64. 8 NeuronCores.

Sharding: core c handles (batch b=c//2, T-half th=c%2) -> 512 query rows.
Each core computes k/v projections for its whole batch (duplicated between
the 2 cores sharing a batch), all 16 heads of attention for its queries and
the full output projection for its rows. Output gather is a pure concat.

Single fused pipeline engineered to keep the PE array continuously busy
(TRN2 p-state: full 2.4 GHz clock only after ~3us of gapless execution):
the attention j-loop for head-pair hp is interleaved with the q/k/v
projections of hp+1 so the PE always has independent matmuls to run while
the DVE(bias-add, in-place in PSUM) -> ACT(exp) chain produces e-tiles.
attn_bias streams as bf16 with one batched DMA per (hp, head, j-half).
Per-head-pair softmax normalization stays on-chip: the two denominators
(65th v-column trick) are reciprocal'd on DVE in-lane, broadcast across
partitions with two K=1 matmuls, and multiplied into oT on GpSimd directly
from PSUM. The output projection runs as a short tail.
"""
import sys

sys.path.insert(0, "/opt/trn_rl_repo")

import numpy as np
import ml_dtypes

import concourse.bass as bass
import concourse.bacc as bacc
import concourse.tile as tile
from concourse import mybir
from concourse.bass_utils import run_bass_kernel_spmd


def _pbcast(ap, nparts):
    """View `ap` (a [1, N] row) replicated across nparts partitions via a
    0-stride partition dim — DMA-source only."""
    row = ap
    return bass.AP(tensor=row.tensor, offset=row.offset,
                   ap=[[0, nparts]] + [list(d) for d in row.ap[1:]])


F32 = mybir.dt.float32
BF16 = mybir.dt.bfloat16
Act = mybir.ActivationFunctionType
Alu = mybir.AluOpType
NPBF16 = ml_dtypes.bfloat16

B, T, S, E, H, D = 4, 1024, 1024, 1024, 16, 64
HP = H // 2          # head pairs
TS = T // 2          # per-core query rows (t-shard)
ET = E // 128        # 128-row tiles of the embed dim
SCALING = D ** -0.5
MASK_NEG = -10000.0

_CACHE = {}


def build_nc():
    nc = bacc.Bacc("TRN2", target_bir_lowering=False, debug=False, num_devices=8)

    qin_d = nc.dram_tensor("qin", [E, TS], BF16, kind="ExternalInput").ap()
    kin_d = nc.dram_tensor("kin", [E, S], BF16, kind="ExternalInput").ap()
    vin_d = nc.dram_tensor("vin", [E, S], BF16, kind="ExternalInput").ap()
    # expb = exp(attn_bias + key_padding_mask*-inf), transposed [H, S, TS]
    bias_d = nc.dram_tensor("expb", [H, S, TS], BF16, kind="ExternalInput").ap()
    wq_d = nc.dram_tensor("wqt", [E, E], BF16, kind="ExternalInput").ap()
    wk_d = nc.dram_tensor("wkt", [E, E], BF16, kind="ExternalInput").ap()
    # wv augmented: per head 64 wv columns + 1 zero column (ones slot)
    wv_d = nc.dram_tensor("wvta", [E, H * 65], BF16, kind="ExternalInput").ap()
    wo_d = nc.dram_tensor("wot", [E, E], BF16, kind="ExternalInput").ap()
    bq_d = nc.dram_tensor("bqs", [128, 8], F32, kind="ExternalInput").ap()
    bk_d = nc.dram_tensor("bks", [128, 8], F32, kind="ExternalInput").ap()
    # per head: 64 bv values + literal 1.0 (fills the ones column)
    bv_d = nc.dram_tensor("bvr65", [1, H * 65], BF16, kind="ExternalInput").ap()
    bo_d = nc.dram_tensor("borb", [1, E], BF16, kind="ExternalInput").ap()
    out_d = nc.dram_tensor("out", [TS, E], F32, kind="ExternalOutput").ap()

    with tile.TileContext(nc) as tc:
        with tc.tile_pool(name="consts", bufs=1) as consts, \
             tc.tile_pool(name="wpool", bufs=1) as wpool, \
             tc.tile_pool(name="in2k", bufs=1) as in2k, \
             tc.tile_pool(name="qpool", bufs=1) as qpool, \
             tc.tile_pool(name="persist", bufs=1) as persist, \
             tc.tile_pool(name="bpool", bufs=12) as bpool, \
             tc.tile_pool(name="estream", bufs=10) as estream, \
             tc.tile_pool(name="eib", bufs=6) as eib, \
             tc.tile_pool(name="osbp", bufs=2) as osbp, \
             tc.tile_pool(name="pscore", bufs=2, space="PSUM") as pscore, \
             tc.tile_pool(name="ppot", bufs=2, space="PSUM") as ppot, \
             tc.tile_pool(name="pproj", bufs=2, space="PSUM") as pproj:

            # ---- constants ----
            bq_sb = consts.tile([128, 8], F32, tag="bq")
            bk_sb = consts.tile([128, 8], F32, tag="bk")
            bvrow = consts.tile([1, H * 65], BF16, tag="bvrow")
            borow = consts.tile([1, E], BF16, tag="borow")  # bo row (bf16)
            ones1 = consts.tile([1, 128], BF16, tag="ones1")
            selAB = consts.tile([1, 256], BF16, tag="selAB")
            selA = selAB[:, 0:128]
            selB = selAB[:, 128:256]
            nc.scalar.dma_start(out=bq_sb, in_=bq_d)
            nc.scalar.dma_start(out=bk_sb, in_=bk_d)
            nc.scalar.dma_start(out=bvrow, in_=bv_d)
            nc.scalar.dma_start(out=borow, in_=bo_d)
            nc.vector.memset(ones1, 1.0)
            nc.vector.memset(selA, 0.0)
            nc.vector.memset(selA[:, 0:64], 1.0)
            nc.vector.memset(selB, 0.0)
            nc.vector.memset(selB[:, 64:128], 1.0)
            # warm the Exp activation table during the DMA-bound prologue
            # (otherwise the first exp pays a ~1.3us table load mid-pipeline)
            warm = consts.tile([1, 8], F32, tag="warm")
            nc.scalar.activation(warm, ones1[:, 0:8], Act.Exp)

            # ---- persistent SBUF tensors ----
            qT = [persist.tile([128, TS], BF16, tag=f"qt{hp}", name=f"qt{hp}")
                  for hp in range(HP)]
            kT = [[persist.tile([128, 512], BF16, tag=f"kt{hp}_{sh}",
                                name=f"kt{hp}_{sh}")
                   for sh in range(2)] for hp in range(HP)]
            # v65[j]: per s-tile j, per head h a 65-wide block: cols 0..63 =
            # d, col 64 = ones (gives the softmax denominator for free as
            # row 64 of the o-matmul PSUM accumulation).
            v65 = [persist.tile([128, H, 65], BF16, tag=f"v65_{j}",
                                name=f"v65_{j}")
                   for j in range(8)]
            otn2 = [persist.tile([128, TS], BF16, tag=f"otn2_{et}",
                                 name=f"otn2_{et}")
                    for et in range(ET)]
            # denominator rows, ACT-copied (with partition shift) to lane 0
            den_f32 = [[persist.tile([1, TS], F32, tag=f"df{hh}_{i}",
                                     name=f"df{hh}_{i}")
                        for hh in range(2)] for i in range(2)]
            rcp_f32 = [[persist.tile([1, TS], F32, tag=f"rf{hh}_{i}",
                                     name=f"rf{hh}_{i}")
                        for hh in range(2)] for i in range(2)]
            rcp_sb = [[persist.tile([1, TS], BF16, tag=f"rcp{hh}_{i}",
                                    name=f"rcp{hh}_{i}")
                       for hh in range(2)] for i in range(2)]
            bc_sb = [persist.tile([128, TS], BF16, tag=f"bc_{i}",
                                  name=f"bc_{i}")
                     for i in range(2)]

            # ---- weights + inputs: ONE consolidated DMA per tensor (the
            # per-DMA DGE config cost would otherwise serialize the
            # prologue). [E, X] dram -> [128, ET*X] sbuf, et-tiles along
            # the free dim.
            def big_load(dram, pool, tag, cols, nsplit=4, eng=None, head=0):
                # split into nsplit DMAs so transfers parallelize across
                # DMA queues; issue from the given sequencer (spreading the
                # per-DMA DGE config cost across idle engines). `head`
                # pulls that many leading columns of every et-tile first so
                # the first consumer unblocks sooner.
                t_ = pool.tile([128, ET * cols], BF16, tag=tag, name=tag)
                src = dram.rearrange("(et p) x -> p et x", p=128)
                dst = t_.rearrange("p (et x) -> p et x", et=ET)
                e_ = eng or nc.sync
                if head:
                    # two queue-parallel head chunks: the first et-half
                    # unblocks the leading accumulation matmuls sooner
                    e_.dma_start(out=dst[:, 0:4, 0:head],
                                 in_=src[:, 0:4, 0:head])
                    e_.dma_start(out=dst[:, 4:ET, 0:head],
                                 in_=src[:, 4:ET, 0:head])

                    def rest(e2=None):
                        (e2 or e_).dma_start(out=dst[:, :, head:cols],
                                             in_=src[:, :, head:cols])
                    return t_, rest
                step = ET // nsplit
                for i in range(nsplit):
                    e_.dma_start(
                        out=dst[:, i * step:(i + 1) * step, :],
                        in_=src[:, i * step:(i + 1) * step, :])
                return t_

            # critical-path chunks first (k/q/v for hp0, s-half 0); the
            # rest streams during hp0's j-loop
            # longest-pole transfers first on each sequencer: kin-head and
            # qin gate k(0,0) / q(0)
            kin_big, kin_rest = big_load(kin_d, in2k, "kin", S, head=512)
            wk_big, wk_rest = big_load(wk_d, wpool, "wk", E, head=256)
            qin_big = big_load(qin_d, qpool, "qin", TS, nsplit=2,
                               eng=nc.scalar)
            wq_big, wq_rest = big_load(wq_d, wpool, "wq", E, eng=nc.scalar,
                                       head=256)
            wv_big, wv_rest = big_load(wv_d, wpool, "wv", H * 65,
                                       eng=nc.gpsimd, head=260)
            vin_big, vin_rest = big_load(vin_d, in2k, "vin", S,
                                         eng=nc.gpsimd, head=512)
            wk_sb = [wk_big[:, et * E:(et + 1) * E] for et in range(ET)]
            wq_sb = [wq_big[:, et * E:(et + 1) * E] for et in range(ET)]
            wv_sb = [wv_big[:, et * H * 65:(et + 1) * H * 65]
                     for et in range(ET)]
            key_sb = [kin_big[:, et * S:(et + 1) * S] for et in range(ET)]
            val_sb = [vin_big[:, et * S:(et + 1) * S] for et in range(ET)]
            qin_sb = [qin_big[:, et * TS:(et + 1) * TS] for et in range(ET)]
            # wo is loaded late (hp6) into the kin buffer — see wo_load()
            wo_sb = []

            def wo_load():
                w = big_load(wo_d, in2k, "kin", E)
                wo_sb.extend(w[:, et * E:(et + 1) * E] for et in range(ET))

            # ---- bias streaming: per (hp, hh, j-quarter) one DMA of 2
            # j-tiles; dest [128, 2*512] bf16, src 3D AP over biasT
            bias_tiles = {}

            def bias_dma(hp, hh, jq, eng=None):
                h = 2 * hp + hh
                bt = bpool.tile([128, 2 * TS], BF16, tag="bias", name="bias")
                src = bias_d[h, jq * 256:(jq + 1) * 256, :].rearrange(
                    "(j p) t -> p j t", p=128)
                (eng or nc.sync).dma_start(
                    out=bt.rearrange("p (j t) -> p j t", t=TS), in_=src)
                bias_tiles[(hp, hh, jq)] = bt

            def bias_slice(hp, hh, j):
                bt = bias_tiles[(hp, hh, j // 2)]
                return bt[:, (j % 2) * TS:(j % 2 + 1) * TS]

            # only the first two j-quarters of hp0's bias up front; the
            # rest goes into hp0's fill (frees early HBM bandwidth for the
            # projection inputs)
            for jq in range(2):
                for hh in range(2):
                    bias_dma(0, hh, jq)

            # ---- projection emitters (one PSUM-group unit each) ----
            def k_unit(hp, sh):
                ps = pproj.tile([128, 512], F32, tag="proj", name="psp")
                for et in range(ET):
                    nc.tensor.matmul(ps, wk_sb[et][:, hp * 128:(hp + 1) * 128],
                                     key_sb[et][:, sh * 512:(sh + 1) * 512],
                                     start=(et == 0), stop=(et == ET - 1))
                nc.vector.tensor_scalar(out=kT[hp][sh], in0=ps,
                                        scalar1=bk_sb[:, hp:hp + 1],
                                        scalar2=None, op0=Alu.add)

            def q_unit(hp):
                ps = pproj.tile([128, 512], F32, tag="proj", name="psp")
                for et in range(ET):
                    nc.tensor.matmul(ps, wq_sb[et][:, hp * 128:(hp + 1) * 128],
                                     qin_sb[et], start=(et == 0),
                                     stop=(et == ET - 1))
                nc.scalar.activation(qT[hp], ps, Act.Identity,
                                     bias=bq_sb[:, hp:hp + 1])

            def v_unit(grp, st):
                # heads 4*grp .. 4*grp+3, s-block st -> v65[st][:, 4 heads].
                # wv is host-augmented with zero columns in the per-head
                # ones slots; the K=1 bv row writes bv plus literal 1.0
                # there, so the PSUM holds finished v65 (contiguous copy).
                ps = pproj.tile([128, 512], F32, tag="proj", name="psp")
                c0 = grp * 260
                for et in range(ET):
                    nc.tensor.matmul(ps[:, 0:260],
                                     val_sb[et][:, st * 128:(st + 1) * 128],
                                     wv_sb[et][:, c0:c0 + 260],
                                     start=(et == 0), stop=False)
                nc.tensor.matmul(ps[:, 0:260], ones1, bvrow[:, c0:c0 + 260],
                                 start=False, stop=True)
                h0 = 4 * grp
                nc.scalar.copy(
                    out=v65[st][:, h0:h0 + 4, :],
                    in_=ps[:, 0:260].rearrange("p (h d) -> p h d", d=65))

            # ---- prologue: hp0 k/q projections; v units and the
            # remaining loads go into hp0's fill so the attention j-loop
            # starts as soon as kT/qT/bias land. kin_rest is issued here —
            # the SP sequencer runs ahead of the PE — so k(0,1) can fill
            # the early DMA-wait hole.
            k_unit(0, 0)
            q_unit(0)
            kin_rest()
            k_unit(0, 1)
            v_unit(0, 0)
            v_unit(0, 1)
            v_unit(0, 2)
            v_unit(0, 3)

            # ---- out-projection group helpers (hp7 early-start + tail)
            og_ps = {}

            def og_mm(g, ps, ets):
                tt, ih = g // 2, g % 2
                for et in ets:
                    nc.tensor.matmul(
                        ps, otn2[et][:, tt * 128:(tt + 1) * 128],
                        wo_sb[et][:, ih * 512:(ih + 1) * 512],
                        start=(et == 0), stop=False)

            def og_start(g):
                # early groups accumulate during hp7 / the normalize-7
                # chain; pool choice avoids circular waits: g0 pproj,
                # g1/g2 pscore, g3 ppot (pot1 slot freed by normalize(6))
                if g == 0:
                    ps = pproj.tile([128, 512], F32, tag="proj", name="psp")
                elif g == 3:
                    ps = ppot.tile([128, 512], F32, tag="pot1", name="pot1")
                else:
                    ps = pscore.tile([128, 512], F32, tag="sc", name="sc")
                og_ps[g] = ps
                og_mm(g, ps, range(6) if g < 2 else range(7))

            # ---- fused attention + interleaved projections ----
            ebuf = {}

            def score_pair(hp, j):
                sh, sl = j // 4, j % 4
                pss = [pscore.tile([128, 512], F32, tag="sc", name="sc")
                       for _ in range(2)]
                for hh in range(2):
                    nc.tensor.matmul(
                        pss[hh],
                        kT[hp][sh][hh * 64:(hh + 1) * 64,
                                   sl * 128:(sl + 1) * 128],
                        qT[hp][hh * 64:(hh + 1) * 64, :],
                        start=True, stop=True,
                        tile_position=(hh * 64, 0))
                return pss

            nmul = [0]

            def bias_exp(hp, j, pss):
                # e = exp(scores) * exp(bias+mask): the exp(bias) factor is
                # host-precomputed, so the elementwise combine is an
                # SBUF-only bf16 multiply that DVE and GpSimd can share.
                for hh in range(2):
                    e0 = eib.tile([128, TS], BF16, tag="ei", name="ei")
                    nc.scalar.activation(e0, pss[hh], Act.Exp)
                    e_ = estream.tile([128, TS], BF16, tag="e", name="e")
                    eng = nc.vector  # bisect: no gpsimd
                    nmul[0] += 1
                    eng.tensor_tensor(out=e_, in0=e0,
                                      in1=bias_slice(hp, hh, j),
                                      op=Alu.mult)
                    ebuf[(j, hh)] = e_

            def o_mm(hp, j, poT):
                for hh in range(2):
                    h = 2 * hp + hh
                    nc.tensor.matmul(poT[hh][0:65, :], v65[j][:, h, :],
                                     ebuf.pop((j, hh)),
                                     start=(j == 0), stop=(j == 7))

            def normalize_rcp(hp, poT):
                # denominators sit on row 64 of each poT (ones-col trick);
                # ACT copy shifts them to lane 0, then fast approx recip.
                i = hp % 2
                for hh in range(2):
                    nc.scalar.copy(out=den_f32[i][hh], in_=poT[hh][64:65, :])
                    nc.vector.reciprocal_approx_fast(out=rcp_f32[i][hh],
                                                     in_=den_f32[i][hh])
                    nc.vector.tensor_copy(out=rcp_sb[i][hh],
                                          in_=rcp_f32[i][hh])

            def normalize_mul(hp, poT, final=False):
                # two K=1 matmuls broadcast 1/den across partitions
                # 0-63 / 64-127, then DVE scales oT into bf16 otn2.
                # The final call draws its PSUM from ppot (pproj/pscore are
                # held by the early out-projection groups then).
                i = hp % 2
                if final:
                    bc = ppot.tile([128, 512], F32, tag="pot0", name="pot0")
                else:
                    bc = pproj.tile([128, 512], F32, tag="proj", name="psp")
                nc.tensor.matmul(bc, selA, rcp_sb[i][0], start=True,
                                 stop=False)
                nc.tensor.matmul(bc, selB, rcp_sb[i][1], start=False,
                                 stop=True)
                nc.scalar.copy(out=bc_sb[i], in_=bc)
                nc.vector.tensor_tensor(out=otn2[hp][0:64, :],
                                        in0=poT[0][0:64, :],
                                        in1=bc_sb[i][0:64, :],
                                        op=Alu.mult)
                nc.vector.tensor_tensor(out=otn2[hp][64:128, :],
                                        in0=poT[1][0:64, :],
                                        in1=bc_sb[i][64:128, :],
                                        op=Alu.mult)

            prev = None  # (hp, poT) awaiting normalize
            for hp in range(HP):
                # fill work: projections + bias prefetch for hp+1, emitted
                # between j-steps so the PE always has independent matmuls
                fill = []
                if hp == 0:
                    for jq in range(2, 4):
                        for hh in range(2):
                            fill.append(
                                lambda hh=hh, jq=jq: bias_dma(0, hh, jq))

                    fill.append(lambda: vin_rest())
                    fill.append(lambda: wk_rest())
                    fill.append(lambda: wq_rest(nc.scalar))
                    fill.append(lambda: wv_rest())
                    for st in range(4, 8):
                        fill.append(lambda st=st: v_unit(0, st))
                if hp + 1 < HP:
                    fill.append(lambda n=hp + 1: k_unit(n, 0))
                    fill.append(lambda n=hp + 1: k_unit(n, 1))
                    fill.append(lambda n=hp + 1: q_unit(n))
                    if hp < 6:
                        g = 1 + hp // 2
                        # grp3's last two s-blocks shift to hp6 (its fill
                        # is otherwise light); they complete well before
                        # hp6's own o_mm consumes those heads
                        if hp == 5:
                            sts = range(4, 6)
                        else:
                            sts = range(0, 4) if hp % 2 == 0 else range(4, 8)
                        for st in sts:
                            fill.append(lambda g=g, st=st: v_unit(g, st))
                    for jq in range(4):
                        for hh in range(2):
                            fill.append(
                                lambda n=hp + 1, hh=hh, jq=jq: bias_dma(n, hh, jq))
                if hp == 6:
                    fill.insert(0, lambda: v_unit(3, 7))
                    fill.insert(0, lambda: v_unit(3, 6))
                    fill.append(wo_load)
                if hp == 7:
                    fill.append(lambda: og_start(0))
                    fill.append(lambda: og_start(1))


                poT = [ppot.tile([128, 512], F32, tag=f"pot{k}", name=f"pot{k}")
                       for k in range(2)]
                nf = len(fill)
                fi = 0
                for j in range(8):
                    pss = score_pair(hp, j)
                    bias_exp(hp, j, pss)
                    if prev is not None:
                        if j == 2:
                            normalize_rcp(*prev)
                        elif j == 5:
                            normalize_mul(*prev)
                    if j >= 2:
                        o_mm(hp, j - 2, poT)
                    # spread fill units across the 8 j-steps
                    want = (j + 1) * nf // 8
                    while fi < want:
                        fill[fi]()
                        fi += 1
                o_mm(hp, 6, poT)
                o_mm(hp, 7, poT)
                prev = (hp, poT)
            normalize_rcp(*prev)
            # keep the PE streaming through the reciprocal chain: groups
            # 2/3 accumulate et 0..6 (otn2[6] settled since hp7 j5)
            og_start(2)
            og_start(3)
            normalize_mul(*prev, final=True)

            # ---- output projection tail: groups 0-1 were started during
            # hp7 (et 0..6); finish them and run groups 2-7 over 4 PSUM
            # banks (pproj + the now-idle pscore pool). bo folds in as a
            # K=1 ones-row, ACT evacuates, DMA out.
            for g in range(8):
                tt, ih = g // 2, g % 2
                if g < 2:
                    ps = og_ps.pop(g)
                    og_mm(g, ps, range(6, ET))
                elif g < 4:
                    ps = og_ps.pop(g)
                    og_mm(g, ps, range(7, ET))
                else:
                    if g % 2 == 0:
                        ps = pproj.tile([128, 512], F32, tag="proj",
                                        name="psp")
                    else:
                        ps = pscore.tile([128, 512], F32, tag="sc",
                                         name="sc")
                    og_mm(g, ps, range(ET))
                nc.tensor.matmul(ps, ones1,
                                 borow[:, ih * 512:(ih + 1) * 512],
                                 start=False, stop=True)
                o = osbp.tile([128, 512], F32, tag="osb", name="osb")
                if g % 2 == 0:
                    nc.scalar.copy(out=o, in_=ps)
                else:
                    nc.vector.tensor_copy(out=o, in_=ps)
                nc.sync.dma_start(
                    out=out_d[tt * 128:(tt + 1) * 128,
                              ih * 512:(ih + 1) * 512],
                    in_=o)

    nc.compile()
    return nc


def _prepare_in_maps(query, key, value, key_padding_mask, attn_bias,
                     wq, bq, wk, bk, wv, bv, wo, bo):
    wqt = (np.ascontiguousarray(wq.T) * SCALING).astype(NPBF16)
    wkt = np.ascontiguousarray(wk.T).astype(NPBF16)
    wot = np.ascontiguousarray(wo.T).astype(NPBF16)
    # wv augmented with a zero column per head (the v65 ones slot)
    wvta = np.zeros((E, H * 65), dtype=NPBF16)
    wvta_v = wvta.reshape(E, H, 65)
    wvta_v[:, :, 0:64] = wv.T.reshape(E, H, 64)
    bvr65 = np.ones((1, H * 65), dtype=NPBF16)
    bvr65.reshape(H, 65)[:, 0:64] = np.asarray(bv, np.float32).reshape(H, 64)
    bqs = np.ascontiguousarray((bq * SCALING).reshape(8, 128).T)
    bks = np.ascontiguousarray(bk.astype(np.float32).reshape(8, 128).T)
    borb = np.ascontiguousarray(bo)[None, :].astype(NPBF16)

    kin_b = [np.ascontiguousarray(key[b_].T).astype(NPBF16) for b_ in range(B)]
    vin_b = [np.ascontiguousarray(value[b_].T).astype(NPBF16) for b_ in range(B)]
    maskadd = [
        np.where(key_padding_mask[b_], MASK_NEG, 0.0).astype(np.float32)
        for b_ in range(B)
    ]

    in_maps = []
    for c in range(8):
        b_, th = c // 2, c % 2
        qin = np.ascontiguousarray(
            query[b_, th * TS:(th + 1) * TS, :].T).astype(NPBF16)
        biasT = np.ascontiguousarray(
            attn_bias[b_ * H:(b_ + 1) * H, th * TS:(th + 1) * TS, :]
            .transpose(0, 2, 1))
        expb = np.exp(biasT + maskadd[b_][None, :, None]).astype(NPBF16)
        in_maps.append({
            "qin": qin, "kin": kin_b[b_], "vin": vin_b[b_],
            "expb": expb,
            "wqt": wqt, "wkt": wkt, "wvta": wvta, "wot": wot,
            "bqs": bqs, "bks": bks, "bvr65": bvr65, "borb": borb,
        })
    return in_maps


def kernel(query, key, value, key_padding_mask, attn_bias,
           wq, bq, wk, bk, wv, bv, wo, bo, _run_kwargs=None):
    query = np.asarray(query, dtype=np.float32)
    key = np.asarray(key, dtype=np.float32)
    value = np.asarray(value, dtype=np.float32)
    key_padding_mask = np.asarray(key_padding_mask)
    attn_bias = np.asarray(attn_bias, dtype=np.float32)
    wq, bq = np.asarray(wq, np.float32), np.asarray(bq, np.float32)
    wk, bk = np.asarray(wk, np.float32), np.asarray(bk, np.float32)
    wv, bv = np.asarray(wv, np.float32), np.asarray(bv, np.float32)
    wo, bo = np.asarray(wo, np.float32), np.asarray(bo, np.float32)

    if "nc" not in _CACHE:
        _CACHE["nc"] = build_nc()
    nc = _CACHE["nc"]

    in_maps = _prepare_in_maps(query, key, value, key_padding_mask, attn_bias,
                               wq, bq, wk, bk, wv, bv, wo, bo)
    res = run_bass_kernel_spmd(nc, in_maps, core_ids=list(range(8)),
                               **(_run_kwargs or {}))
    _CACHE["last_results"] = res

    out = np.empty((B, T, E), dtype=np.float32)
    for c in range(8):
        b_, th = c // 2, c % 2
        out[b_, th * TS:(th + 1) * TS, :] = res.results[c]["out"]
    return out
